# revision 9
# baseline (speedup 1.0000x reference)
"""APLoss distributed Bass kernel for 8 TRN2 NeuronCores.

Reference math, restructured with an indicator decomposition:
    sur[i,j] = relu(t)^2,  t = negf_i + y_j,  negf_i = MARGIN - f_i
    relu(t)^2 = t^2 * H,   H = 1[t > 0]
    S_i = sum_j sur = negf_i^2 * A_i + 2*negf_i * B_i + C_i
      where A_i = sum_j H_ij, B_i = sum_j H_ij*y_j, C_i = sum_j H_ij*y_j^2
    T_i = masked version with (Am, Bm, Cm) using weights m_j*[1, y, y^2]
    ua_i = (1-g)*u_all[index_p[i]] + g*S_i/N
    up_i = (1-g)*u_pos[index_p[i]] + g*T_i/N
    loss = sum_i (up_i*S_i - ua_i*T_i) / ua_i^2 / (P*N)

Sharding: rows (positives) split 8 ways, 256 rows/core; y replicated.
Device layout: columns j on partitions (128 j-blocks of 128), rows i on
the free axis (256). Per core:
  DVE  (88 blocks): H = (negf + y_j) > 0           (tensor_scalar add,is_gt)
  ACT  (40 blocks): Hs = Sign(negf + y_j)          (activation, bias=y_j)
  PE: W_b^T @ H_b with W_b = [1, y, y^2, m, m*y, m*y^2] (host-built, bf16),
      4-way column-tiled (tile_position) into psumH/psumS row-groups at
      partitions {0,32,64,96}, accumulated over blocks.
  Sign-block sums corrected on device: H.W = (Hs.W + sum(W))/2 with sum(W)
  over the ACT column range passed as host constants.
  Finalize transposed to [128,2] (rows on partitions) for cheap vector ops;
  per-core scalar partial out; host sums the 8 partials.
"""

import sys

if "/opt/trn_rl_repo" not in sys.path:
    sys.path.insert(0, "/opt/trn_rl_repo")

import ml_dtypes
import numpy as np

import concourse.bass as bass
import concourse.tile as tile
from concourse import bacc, mybir
from concourse import bass_utils
from concourse.masks import make_identity

N = 16384
P = 2048
N_CORES = 8
PC = P // N_CORES          # rows per core (free dim)
JB = 128                   # j-block size (partitions)
NB = N // JB               # number of j-blocks
NH = PC // JB              # halves of the row range (2)
GAMMA = 0.99
MARGIN = 1.0
INV_PN = 1.0 / (P * N)     # 2^-25, exact

DVE_BLOCKS = 88            # H-blocks on the vector engine; rest on scalar
NG = 4                     # PE column-tile groups

TRACE = False
LAST_RESULT = None

_COMPILED = {}

f32 = mybir.dt.float32
bf16 = mybir.dt.bfloat16
Alu = mybir.AluOpType
Act = mybir.ActivationFunctionType
bfnp = ml_dtypes.bfloat16


def _build():
    nc = bacc.Bacc("TRN2", target_bir_lowering=False, debug=False,
                   num_devices=N_CORES)

    yb_d = nc.dram_tensor("yb", [JB, NB], f32, kind="ExternalInput")
    w_d = nc.dram_tensor("W", [JB, NB * 6], bf16, kind="ExternalInput")
    negfb_d = nc.dram_tensor("negfb", [JB, PC], bf16, kind="ExternalInput")
    negfT_d = nc.dram_tensor("negfT", [JB, NH], f32, kind="ExternalInput")
    uallT_d = nc.dram_tensor("uallT", [JB, NH], f32, kind="ExternalInput")
    uposT_d = nc.dram_tensor("uposT", [JB, NH], f32, kind="ExternalInput")
    corrb_d = nc.dram_tensor("corrb", [JB, 6], f32, kind="ExternalInput")
    out_d = nc.dram_tensor("out", [1, 1], f32, kind="ExternalOutput")

    with tile.TileContext(nc) as tc:
        with (
            tc.tile_pool(name="const", bufs=1) as cpool,
            tc.tile_pool(name="hpool", bufs=NB) as hpool,
            tc.tile_pool(name="psum", bufs=1, space="PSUM") as ppool,
            tc.tile_pool(name="small", bufs=1) as spool,
        ):
            y_f32 = cpool.tile([JB, NB], f32, name="y_f32")
            nc.sync.dma_start(y_f32[:], yb_d[:])
            W_all = cpool.tile([JB, NB * 6], bf16, name="W_all")
            nc.sync.dma_start(W_all[:], w_d[:])
            negf_bf = cpool.tile([JB, PC], bf16, name="negf_bf")
            nc.sync.dma_start(negf_bf[:], negfb_d[:])

            negfT = spool.tile([JB, NH], f32, name="negfT")
            nc.sync.dma_start(negfT[:], negfT_d[:])
            uallT = spool.tile([JB, NH], f32, name="uallT")
            nc.sync.dma_start(uallT[:], uallT_d[:])
            uposT = spool.tile([JB, NH], f32, name="uposT")
            nc.sync.dma_start(uposT[:], uposT_d[:])
            corrb = spool.tile([JB, 6], f32, name="corrb")
            nc.sync.dma_start(corrb[:], corrb_d[:])

            ident = cpool.tile([JB, JB], f32, name="ident")
            make_identity(nc, ident)
            ones_f = cpool.tile([JB, 1], f32, name="ones_f")
            nc.vector.memset(ones_f[:], 1.0)

            # ---- H pass ----
            h_tiles = []
            for b in range(NB):
                h = hpool.tile([JB, PC], bf16, name=f"h{b}", tag="h")
                if b < DVE_BLOCKS:
                    nc.vector.tensor_scalar(h[:], negf_bf[:],
                                            y_f32[:, b:b + 1], 0.0,
                                            Alu.add, Alu.is_gt)
                else:
                    nc.scalar.activation(h[:], negf_bf[:], Act.Sign,
                                         bias=y_f32[:, b:b + 1])
                h_tiles.append(h)

            # ---- PE contraction, 4-way column-tiled ----
            # one PSUM bank per column-group so Tile's bank-overlap
            # tracking does not serialize the concurrent strips
            psumHg = [ppool.tile([JB, PC], f32, name=f"psumHg{g}",
                                 tag=f"pg{g}") for g in range(NG)]
            psumSg = [ppool.tile([JB, PC], f32, name=f"psumSg{g}",
                                 tag=f"pg{NG + g}") for g in range(NG)]
            h_last = {}
            for b in range(NB):
                which = b < DVE_BLOCKS
                g = b % NG
                h_last[(which, g)] = b
            seen = set()
            for b in range(NB):
                which = b < DVE_BLOCKS
                g = b % NG
                acc = psumHg[g] if which else psumSg[g]
                first = (which, g) not in seen
                seen.add((which, g))
                last = h_last[(which, g)] == b
                nc.tensor.matmul(acc[32 * g:32 * g + 6, :],
                                 W_all[:, b * 6:(b + 1) * 6],
                                 h_tiles[b][:], start=first, stop=last,
                                 tile_position=(0, 32 * g),
                                 skip_group_check=True)

            # ---- merge the 4 column-groups (one PSUM operand per op) ----
            Hsb = spool.tile([6, PC], f32, name="Hsb")
            nc.vector.tensor_copy(Hsb[:], psumHg[0][0:6, :])
            nc.vector.tensor_add(Hsb[:], Hsb[:], psumHg[1][32:38, :])
            nc.vector.tensor_add(Hsb[:], Hsb[:], psumHg[2][64:70, :])
            nc.vector.tensor_add(Hsb[:], Hsb[:], psumHg[3][96:102, :])

            Ssb = spool.tile([6, PC], f32, name="Ssb")
            nc.vector.tensor_copy(Ssb[:], psumSg[0][0:6, :])
            nc.vector.tensor_add(Ssb[:], Ssb[:], psumSg[1][32:38, :])
            nc.vector.tensor_add(Ssb[:], Ssb[:], psumSg[2][64:70, :])
            nc.vector.tensor_add(Ssb[:], Ssb[:], psumSg[3][96:102, :])

            contrib = spool.tile([JB, NH], f32, name="contrib")
            psumT = ppool.tile([JB, NH * 6], f32, name="psumT", tag="pg0")
            psumT2 = ppool.tile([JB, NH * 6], f32, name="psumT2", tag="pg1")
            for hh in range(NH):
                nc.tensor.transpose(psumT[:, hh * 6:(hh + 1) * 6],
                                    Hsb[:, hh * JB:(hh + 1) * JB],
                                    ident[0:6, 0:6])
                nc.tensor.transpose(psumT2[:, hh * 6:(hh + 1) * 6],
                                    Ssb[:, hh * JB:(hh + 1) * JB],
                                    ident[0:6, 0:6])

            for hh in range(NH):
                VH = spool.tile([JB, 6], f32, name=f"VH{hh}", tag="VH")
                nc.vector.tensor_copy(VH[:], psumT[:, hh * 6:(hh + 1) * 6])
                VS = spool.tile([JB, 6], f32, name=f"VS{hh}", tag="VS")
                nc.vector.tensor_copy(VS[:], psumT2[:, hh * 6:(hh + 1) * 6])

                # Vc = VH + 0.5*(VS + corr)  -> [A,B,C,Am,Bm,Cm]
                Vc = spool.tile([JB, 6], f32, name=f"Vc{hh}", tag="Vc")
                nc.vector.tensor_add(Vc[:], VS[:], corrb[:])
                nc.vector.scalar_tensor_tensor(Vc[:], Vc[:], 0.5, VH[:],
                                               Alu.mult, Alu.add)

                nf = negfT[:, hh:hh + 1]
                nf2 = spool.tile([JB, 1], f32, name=f"nf2{hh}", tag="nf2")
                nc.vector.tensor_mul(nf2[:], nf, nf)
                nf_2 = spool.tile([JB, 1], f32, name=f"nf_2{hh}", tag="nf_2")
                nc.vector.tensor_scalar(nf_2[:], nf, 2.0, 0.0,
                                        Alu.mult, Alu.add)

                # S = negf^2*A + (2negf*B + C); T likewise on masked cols
                S = spool.tile([JB, 1], f32, name=f"S{hh}", tag="S")
                nc.vector.scalar_tensor_tensor(S[:], Vc[:, 1:2], nf_2[:],
                                               Vc[:, 2:3], Alu.mult, Alu.add)
                nc.vector.scalar_tensor_tensor(S[:], Vc[:, 0:1], nf2[:],
                                               S[:], Alu.mult, Alu.add)
                T = spool.tile([JB, 1], f32, name=f"T{hh}", tag="T")
                nc.vector.scalar_tensor_tensor(T[:], Vc[:, 4:5], nf_2[:],
                                               Vc[:, 5:6], Alu.mult, Alu.add)
                nc.vector.scalar_tensor_tensor(T[:], Vc[:, 3:4], nf2[:],
                                               T[:], Alu.mult, Alu.add)

                # ua = (1-g)*uall + (g/N)*S ; up = (1-g)*upos + (g/N)*T
                ua = spool.tile([JB, 1], f32, name=f"ua{hh}", tag="ua")
                nc.vector.tensor_scalar(ua[:], uallT[:, hh:hh + 1],
                                        1.0 - GAMMA, 0.0, Alu.mult, Alu.add)
                nc.vector.scalar_tensor_tensor(ua[:], S[:], GAMMA / N, ua[:],
                                               Alu.mult, Alu.add)
                up = spool.tile([JB, 1], f32, name=f"up{hh}", tag="up")
                nc.vector.tensor_scalar(up[:], uposT[:, hh:hh + 1],
                                        1.0 - GAMMA, 0.0, Alu.mult, Alu.add)
                nc.vector.scalar_tensor_tensor(up[:], T[:], GAMMA / N, up[:],
                                               Alu.mult, Alu.add)

                inv = spool.tile([JB, 1], f32, name=f"inv{hh}", tag="inv")
                nc.vector.reciprocal(inv[:], ua[:])

                t1 = spool.tile([JB, 1], f32, name=f"t1{hh}", tag="t1")
                nc.vector.tensor_mul(t1[:], up[:], S[:])
                t2 = spool.tile([JB, 1], f32, name=f"t2{hh}", tag="t2")
                nc.vector.tensor_mul(t2[:], ua[:], T[:])
                d = spool.tile([JB, 1], f32, name=f"d{hh}", tag="d")
                nc.vector.tensor_sub(d[:], t1[:], t2[:])
                nc.vector.tensor_mul(d[:], d[:], inv[:])
                nc.vector.tensor_mul(d[:], d[:], inv[:])
                nc.vector.tensor_copy(contrib[:, hh:hh + 1], d[:])

            csum = spool.tile([JB, 1], f32, name="csum")
            nc.vector.tensor_add(csum[:], contrib[:, 0:1], contrib[:, 1:2])
            psum1 = ppool.tile([1, 1], f32, name="psum1", tag="pg2")
            nc.tensor.matmul(psum1[:], ones_f[:], csum[:], start=True,
                             stop=True)
            partial = spool.tile([1, 1], f32, name="partial")
            nc.vector.tensor_scalar(partial[:], psum1[:], INV_PN, 0.0,
                                    Alu.mult, Alu.add)
            nc.sync.dma_start(out_d[:], partial[:])

    nc.compile()
    return nc


def _host_w(yb: np.ndarray, maskb: np.ndarray):
    """W[p, b, :] = [1, y, y^2, m, m*y, m*y^2] in bf16, plus f64 copy."""
    y = yb.astype(np.float32)
    y2 = (y * y).astype(np.float32)
    m = maskb.astype(np.float32)
    w = np.stack([np.ones_like(y), y, y2, m, m * y, m * y2], axis=-1)
    wb = w.astype(bfnp)
    return np.ascontiguousarray(wb.reshape(JB, NB * 6))


def kernel(y_pred, y_true, index_p, pos_idx, u_all, u_pos):
    global LAST_RESULT

    yp = np.asarray(y_pred, dtype=np.float32).reshape(-1)
    maskf = (np.asarray(y_true, dtype=np.float32).reshape(-1) == 1.0
             ).astype(np.float32)
    index_p = np.asarray(index_p).reshape(-1)
    pos_idx = np.asarray(pos_idx).reshape(-1)
    u_all_b = np.asarray(u_all, dtype=np.float32).reshape(-1)[index_p]
    u_pos_b = np.asarray(u_pos, dtype=np.float32).reshape(-1)[index_p]

    f_ps = yp[pos_idx]
    negf = (MARGIN - f_ps).astype(np.float32)       # (P,)

    nc = _COMPILED.get("nc")
    if nc is None:
        nc = _build()
        _COMPILED["nc"] = nc

    yb = np.ascontiguousarray(yp.reshape(NB, JB).T)
    maskb = np.ascontiguousarray(maskf.reshape(NB, JB).T)
    W = _host_w(yb, maskb)

    # sums of the (bf16-rounded) W columns over the ACT block range, for
    # the sign correction H.W = (Hs.W + sum(W))/2
    Wf = W.reshape(JB, NB, 6).astype(np.float64)
    corr = Wf[:, DVE_BLOCKS:, :].sum(axis=(0, 1)).astype(np.float32)
    corrb = np.ascontiguousarray(
        np.broadcast_to(corr, (JB, 6))).astype(np.float32)

    in_maps = []
    for c in range(N_CORES):
        rs = slice(c * PC, (c + 1) * PC)
        negf_c = negf[rs]
        in_maps.append({
            "yb": yb,
            "W": W,
            "negfb": np.ascontiguousarray(
                np.broadcast_to(negf_c, (JB, PC))).astype(bfnp),
            "negfT": np.ascontiguousarray(negf_c.reshape(NH, JB).T),
            "uallT": np.ascontiguousarray(
                u_all_b[rs].reshape(NH, JB).T).astype(np.float32),
            "uposT": np.ascontiguousarray(
                u_pos_b[rs].reshape(NH, JB).T).astype(np.float32),
            "corrb": corrb,
        })

    res = bass_utils.run_bass_kernel_spmd(
        nc, in_maps, core_ids=list(range(N_CORES)), trace=TRACE)
    LAST_RESULT = res

    total = np.float32(0.0)
    for c in range(N_CORES):
        total = np.float32(total + res.results[c]["out"][0, 0])
    return np.asarray(total, dtype=np.float32)


# revision 12
# speedup vs baseline: 1.1855x; 1.1855x over previous
"""APLoss distributed Bass kernel for 8 TRN2 NeuronCores.

Reference math, restructured with an indicator decomposition:
    sur[i,j] = relu(t)^2,  t = negf_i + y_j,  negf_i = MARGIN - f_i
    relu(t)^2 = t^2 * H,   H = 1[t > 0]
    S_i = sum_j sur = negf_i^2 * A_i + 2*negf_i * B_i + C_i
      where A_i = sum_j H_ij, B_i = sum_j H_ij*y_j, C_i = sum_j H_ij*y_j^2
    T_i = masked version with (Am, Bm, Cm) using weights m_j*[1, y, y^2]
    ua_i = (1-g)*u_all[index_p[i]] + g*S_i/N
    up_i = (1-g)*u_pos[index_p[i]] + g*T_i/N
    loss = sum_i (up_i*S_i - ua_i*T_i) / ua_i^2 / (P*N)

Sharding: rows (positives) split 8 ways, 256 rows/core; y replicated.
Device layout: columns j on partitions (128 j-blocks of 128), rows i on
the free axis (256). Per core:
  DVE  (88 blocks): H = (negf + y_j) > 0           (tensor_scalar add,is_gt)
  ACT  (40 blocks): Hs = Sign(negf + y_j)          (activation, bias=y_j)
  PE: W_b^T @ H_b with W_b = [1, y, y^2, m, m*y, m*y^2] (host-built, bf16),
      4-way column-tiled (tile_position) into psumH/psumS row-groups at
      partitions {0,32,64,96}, accumulated over blocks.
  Sign-block sums corrected on device: H.W = (Hs.W + sum(W))/2 with sum(W)
  over the ACT column range passed as host constants.
  Finalize transposed to [128,2] (rows on partitions) for cheap vector ops;
  per-core scalar partial out; host sums the 8 partials.
"""

import sys

if "/opt/trn_rl_repo" not in sys.path:
    sys.path.insert(0, "/opt/trn_rl_repo")

import ml_dtypes
import numpy as np

import concourse.bass as bass
import concourse.tile as tile
from concourse import bacc, mybir
from concourse import bass_utils
from concourse.masks import make_identity

N = 16384
P = 2048
N_CORES = 8
PC = P // N_CORES          # rows per core (free dim)
JB = 128                   # j-block size (partitions)
NB = N // JB               # number of j-blocks
NH = PC // JB              # halves of the row range (2)
GAMMA = 0.99
MARGIN = 1.0
INV_PN = 1.0 / (P * N)     # 2^-25, exact

DVE_BLOCKS = 88            # H-blocks on the vector engine; rest on scalar (even)
WK = 16                    # padded stationary columns per block (fp8 DoubleRow
                           # needs 16-byte steps between K-tiles)

TRACE = False
LAST_RESULT = None

_COMPILED = {}
LDW_OPT = False            # walrus --enable-ldw-opt=true crashes codegen

_orig_run_command = bass_utils.run_command


def _run_command_ldw(argv, **kwargs):
    if LDW_OPT:
        argv = ["--enable-ldw-opt=true" if a == "--enable-ldw-opt=false"
                else a for a in argv]
    return _orig_run_command(argv, **kwargs)


bass_utils.run_command = _run_command_ldw

f32 = mybir.dt.float32
bf16 = mybir.dt.bfloat16
f8 = mybir.dt.float8e4
Alu = mybir.AluOpType
Act = mybir.ActivationFunctionType
bfnp = ml_dtypes.bfloat16
f8np = ml_dtypes.float8_e4m3


def _build():
    nc = bacc.Bacc("TRN2", target_bir_lowering=False, debug=False,
                   num_devices=N_CORES)

    yb_d = nc.dram_tensor("yb", [JB, NB], f32, kind="ExternalInput")
    w_d = nc.dram_tensor("W", [JB, NB * WK], f8, kind="ExternalInput")
    negfb_d = nc.dram_tensor("negfb", [JB, PC], bf16, kind="ExternalInput")
    negfT_d = nc.dram_tensor("negfT", [JB, NH], f32, kind="ExternalInput")
    uallT_d = nc.dram_tensor("uallT", [JB, NH], f32, kind="ExternalInput")
    uposT_d = nc.dram_tensor("uposT", [JB, NH], f32, kind="ExternalInput")
    corrb_d = nc.dram_tensor("corrb", [JB, 6], f32, kind="ExternalInput")
    out_d = nc.dram_tensor("out", [1, 1], f32, kind="ExternalOutput")

    with tile.TileContext(nc) as tc:
        with (
            tc.tile_pool(name="const", bufs=1) as cpool,
            tc.tile_pool(name="hpool", bufs=NB) as hpool,
            tc.tile_pool(name="psum", bufs=1, space="PSUM") as ppool,
            tc.tile_pool(name="small", bufs=1) as spool,
        ):
            y_f32 = cpool.tile([JB, NB], f32, name="y_f32")
            nc.sync.dma_start(y_f32[:], yb_d[:])
            negf_bf = cpool.tile([JB, PC], bf16, name="negf_bf")
            nc.sync.dma_start(negf_bf[:], negfb_d[:])
            W_all = cpool.tile([JB, NB * WK], f8, name="W_all")
            nc.sync.dma_start(W_all[:], w_d[:])

            negfT = spool.tile([JB, NH], f32, name="negfT")
            nc.sync.dma_start(negfT[:], negfT_d[:])
            uallT = spool.tile([JB, NH], f32, name="uallT")
            nc.sync.dma_start(uallT[:], uallT_d[:])
            uposT = spool.tile([JB, NH], f32, name="uposT")
            nc.sync.dma_start(uposT[:], uposT_d[:])
            corrb = spool.tile([JB, 6], f32, name="corrb")
            nc.sync.dma_start(corrb[:], corrb_d[:])

            ident = cpool.tile([JB, JB], f32, name="ident")
            make_identity(nc, ident)
            ones_f = cpool.tile([JB, 1], f32, name="ones_f")
            nc.vector.memset(ones_f[:], 1.0)

            # ---- H pass (into one fp8 tile; 0/1/-1 are exact in fp8) ----
            H_all = cpool.tile([JB, NB * PC], f8, name="H_all")
            for b in range(NB):
                hsl = H_all[:, b * PC:(b + 1) * PC]
                if b < DVE_BLOCKS:
                    nc.vector.tensor_scalar(hsl, negf_bf[:],
                                            y_f32[:, b:b + 1], 0.0,
                                            Alu.add, Alu.is_gt)
                else:
                    nc.scalar.activation(hsl, negf_bf[:], Act.Sign,
                                         bias=y_f32[:, b:b + 1])

            # ---- PE contraction: fp8 DoubleRow, 2 j-blocks per matmul ----
            psumH = ppool.tile([JB, PC], f32, name="psumH", tag="pg0")
            psumS = ppool.tile([JB, PC], f32, name="psumS", tag="pg1")
            w3d = W_all[:].rearrange("p (b k) -> p b k", k=WK)
            h3d = H_all[:].rearrange("p (b i) -> p b i", i=PC)
            NPAIR = NB // 2
            DPAIR = DVE_BLOCKS // 2
            for pr in range(NPAIR):
                which = pr < DPAIR
                acc = psumH if which else psumS
                first = pr == 0 or pr == DPAIR
                last = pr == DPAIR - 1 or pr == NPAIR - 1
                nc.tensor.matmul(acc[0:WK, :],
                                 w3d[:, 2 * pr:2 * pr + 2, :],
                                 h3d[:, 2 * pr:2 * pr + 2, :],
                                 start=first, stop=last,
                                 perf_mode=mybir.MatmulPerfMode.DoubleRow)

            Hsb = spool.tile([6, PC], f32, name="Hsb")
            nc.vector.tensor_copy(Hsb[:], psumH[0:6, :])
            Ssb = spool.tile([6, PC], f32, name="Ssb")
            nc.vector.tensor_copy(Ssb[:], psumS[0:6, :])

            contrib = spool.tile([JB, NH], f32, name="contrib")
            psumT = ppool.tile([JB, NH * 6], f32, name="psumT", tag="pg0")
            psumT2 = ppool.tile([JB, NH * 6], f32, name="psumT2", tag="pg1")
            for hh in range(NH):
                nc.tensor.transpose(psumT[:, hh * 6:(hh + 1) * 6],
                                    Hsb[:, hh * JB:(hh + 1) * JB],
                                    ident[0:6, 0:6])
                nc.tensor.transpose(psumT2[:, hh * 6:(hh + 1) * 6],
                                    Ssb[:, hh * JB:(hh + 1) * JB],
                                    ident[0:6, 0:6])

            for hh in range(NH):
                VH = spool.tile([JB, 6], f32, name=f"VH{hh}", tag="VH")
                nc.vector.tensor_copy(VH[:], psumT[:, hh * 6:(hh + 1) * 6])
                VS = spool.tile([JB, 6], f32, name=f"VS{hh}", tag="VS")
                nc.vector.tensor_copy(VS[:], psumT2[:, hh * 6:(hh + 1) * 6])

                # Vc = VH + 0.5*(VS + corr)  -> [A,B,C,Am,Bm,Cm]
                Vc = spool.tile([JB, 6], f32, name=f"Vc{hh}", tag="Vc")
                nc.vector.tensor_add(Vc[:], VS[:], corrb[:])
                nc.vector.scalar_tensor_tensor(Vc[:], Vc[:], 0.5, VH[:],
                                               Alu.mult, Alu.add)

                nf = negfT[:, hh:hh + 1]
                nf2 = spool.tile([JB, 1], f32, name=f"nf2{hh}", tag="nf2")
                nc.vector.tensor_mul(nf2[:], nf, nf)
                nf_2 = spool.tile([JB, 1], f32, name=f"nf_2{hh}", tag="nf_2")
                nc.vector.tensor_scalar(nf_2[:], nf, 2.0, 0.0,
                                        Alu.mult, Alu.add)

                # S = negf^2*A + (2negf*B + C); T likewise on masked cols
                S = spool.tile([JB, 1], f32, name=f"S{hh}", tag="S")
                nc.vector.scalar_tensor_tensor(S[:], Vc[:, 1:2], nf_2[:],
                                               Vc[:, 2:3], Alu.mult, Alu.add)
                nc.vector.scalar_tensor_tensor(S[:], Vc[:, 0:1], nf2[:],
                                               S[:], Alu.mult, Alu.add)
                T = spool.tile([JB, 1], f32, name=f"T{hh}", tag="T")
                nc.vector.scalar_tensor_tensor(T[:], Vc[:, 4:5], nf_2[:],
                                               Vc[:, 5:6], Alu.mult, Alu.add)
                nc.vector.scalar_tensor_tensor(T[:], Vc[:, 3:4], nf2[:],
                                               T[:], Alu.mult, Alu.add)

                # ua = (1-g)*uall + (g/N)*S ; up = (1-g)*upos + (g/N)*T
                ua = spool.tile([JB, 1], f32, name=f"ua{hh}", tag="ua")
                nc.vector.tensor_scalar(ua[:], uallT[:, hh:hh + 1],
                                        1.0 - GAMMA, 0.0, Alu.mult, Alu.add)
                nc.vector.scalar_tensor_tensor(ua[:], S[:], GAMMA / N, ua[:],
                                               Alu.mult, Alu.add)
                up = spool.tile([JB, 1], f32, name=f"up{hh}", tag="up")
                nc.vector.tensor_scalar(up[:], uposT[:, hh:hh + 1],
                                        1.0 - GAMMA, 0.0, Alu.mult, Alu.add)
                nc.vector.scalar_tensor_tensor(up[:], T[:], GAMMA / N, up[:],
                                               Alu.mult, Alu.add)

                inv = spool.tile([JB, 1], f32, name=f"inv{hh}", tag="inv")
                nc.vector.reciprocal(inv[:], ua[:])

                t1 = spool.tile([JB, 1], f32, name=f"t1{hh}", tag="t1")
                nc.vector.tensor_mul(t1[:], up[:], S[:])
                t2 = spool.tile([JB, 1], f32, name=f"t2{hh}", tag="t2")
                nc.vector.tensor_mul(t2[:], ua[:], T[:])
                d = spool.tile([JB, 1], f32, name=f"d{hh}", tag="d")
                nc.vector.tensor_sub(d[:], t1[:], t2[:])
                nc.vector.tensor_mul(d[:], d[:], inv[:])
                nc.vector.tensor_mul(d[:], d[:], inv[:])
                nc.vector.tensor_copy(contrib[:, hh:hh + 1], d[:])

            csum = spool.tile([JB, 1], f32, name="csum")
            nc.vector.tensor_add(csum[:], contrib[:, 0:1], contrib[:, 1:2])
            psum1 = ppool.tile([1, 1], f32, name="psum1", tag="pg2")
            nc.tensor.matmul(psum1[:], ones_f[:], csum[:], start=True,
                             stop=True)
            partial = spool.tile([1, 1], f32, name="partial")
            nc.vector.tensor_scalar(partial[:], psum1[:], INV_PN, 0.0,
                                    Alu.mult, Alu.add)
            nc.sync.dma_start(out_d[:], partial[:])

    nc.compile()
    return nc


def _host_w(yb: np.ndarray, maskb: np.ndarray):
    """W[p, b, :] = [1, y, y^2, m, m*y, m*y^2, 0...] in fp8e4 (WK cols)."""
    y = yb.astype(np.float32)
    y2 = (y * y).astype(np.float32)
    m = maskb.astype(np.float32)
    w = np.zeros((JB, NB, WK), dtype=np.float32)
    w[:, :, 0] = 1.0
    w[:, :, 1] = y
    w[:, :, 2] = y2
    w[:, :, 3] = m
    w[:, :, 4] = m * y
    w[:, :, 5] = m * y2
    wb = w.astype(f8np)
    return np.ascontiguousarray(wb.reshape(JB, NB * WK))


def kernel(y_pred, y_true, index_p, pos_idx, u_all, u_pos):
    global LAST_RESULT

    yp = np.asarray(y_pred, dtype=np.float32).reshape(-1)
    maskf = (np.asarray(y_true, dtype=np.float32).reshape(-1) == 1.0
             ).astype(np.float32)
    index_p = np.asarray(index_p).reshape(-1)
    pos_idx = np.asarray(pos_idx).reshape(-1)
    u_all_b = np.asarray(u_all, dtype=np.float32).reshape(-1)[index_p]
    u_pos_b = np.asarray(u_pos, dtype=np.float32).reshape(-1)[index_p]

    f_ps = yp[pos_idx]
    negf = (MARGIN - f_ps).astype(np.float32)       # (P,)

    nc = _COMPILED.get("nc")
    if nc is None:
        nc = _build()
        _COMPILED["nc"] = nc

    yb = np.ascontiguousarray(yp.reshape(NB, JB).T)
    maskb = np.ascontiguousarray(maskf.reshape(NB, JB).T)
    W = _host_w(yb, maskb)

    # sums of the (bf16-rounded) W columns over the ACT block range, for
    # the sign correction H.W = (Hs.W + sum(W))/2
    Wf = W.reshape(JB, NB, WK).astype(np.float64)
    corr = Wf[:, DVE_BLOCKS:, :6].sum(axis=(0, 1)).astype(np.float32)
    corrb = np.ascontiguousarray(
        np.broadcast_to(corr, (JB, 6))).astype(np.float32)

    in_maps = []
    for c in range(N_CORES):
        rs = slice(c * PC, (c + 1) * PC)
        negf_c = negf[rs]
        in_maps.append({
            "yb": yb,
            "W": W,
            "negfb": np.ascontiguousarray(
                np.broadcast_to(negf_c, (JB, PC))).astype(bfnp),
            "negfT": np.ascontiguousarray(negf_c.reshape(NH, JB).T),
            "uallT": np.ascontiguousarray(
                u_all_b[rs].reshape(NH, JB).T).astype(np.float32),
            "uposT": np.ascontiguousarray(
                u_pos_b[rs].reshape(NH, JB).T).astype(np.float32),
            "corrb": corrb,
        })

    res = bass_utils.run_bass_kernel_spmd(
        nc, in_maps, core_ids=list(range(N_CORES)), trace=TRACE)
    LAST_RESULT = res

    total = np.float32(0.0)
    for c in range(N_CORES):
        total = np.float32(total + res.results[c]["out"][0, 0])
    return np.asarray(total, dtype=np.float32)


# revision 13
# speedup vs baseline: 1.1902x; 1.0039x over previous
"""APLoss distributed Bass kernel for 8 TRN2 NeuronCores.

Reference math, restructured with an indicator decomposition:
    sur[i,j] = relu(t)^2,  t = negf_i + y_j,  negf_i = MARGIN - f_i
    relu(t)^2 = t^2 * H,   H = 1[t > 0]
    S_i = sum_j sur = negf_i^2 * A_i + 2*negf_i * B_i + C_i
      where A_i = sum_j H_ij, B_i = sum_j H_ij*y_j, C_i = sum_j H_ij*y_j^2
    T_i = masked version with (Am, Bm, Cm) using weights m_j*[1, y, y^2]
    ua_i = (1-g)*u_all[index_p[i]] + g*S_i/N
    up_i = (1-g)*u_pos[index_p[i]] + g*T_i/N
    loss = sum_i (up_i*S_i - ua_i*T_i) / ua_i^2 / (P*N)

Sharding: rows (positives) split 8 ways, 256 rows/core; y replicated.
Device layout: columns j on partitions (128 j-blocks of 128), rows i on
the free axis (256). Per core:
  DVE  (88 blocks): H = (negf + y_j) > 0           (tensor_scalar add,is_gt)
  ACT  (40 blocks): Hs = Sign(negf + y_j)          (activation, bias=y_j)
  PE: W_b^T @ H_b with W_b = [1, y, y^2, m, m*y, m*y^2] (host-built, bf16),
      4-way column-tiled (tile_position) into psumH/psumS row-groups at
      partitions {0,32,64,96}, accumulated over blocks.
  Sign-block sums corrected on device: H.W = (Hs.W + sum(W))/2 with sum(W)
  over the ACT column range passed as host constants.
  Finalize transposed to [128,2] (rows on partitions) for cheap vector ops;
  per-core scalar partial out; host sums the 8 partials.
"""

import sys

if "/opt/trn_rl_repo" not in sys.path:
    sys.path.insert(0, "/opt/trn_rl_repo")

import ml_dtypes
import numpy as np

import concourse.bass as bass
import concourse.tile as tile
from concourse import bacc, mybir
from concourse import bass_utils
from concourse.masks import make_identity

N = 16384
P = 2048
N_CORES = 8
PC = P // N_CORES          # rows per core (free dim)
JB = 128                   # j-block size (partitions)
NB = N // JB               # number of j-blocks
NH = PC // JB              # halves of the row range (2)
GAMMA = 0.99
MARGIN = 1.0
INV_PN = 1.0 / (P * N)     # 2^-25, exact

DVE_BLOCKS = 88            # H-blocks on the vector engine; rest on scalar (even)
WK = 16                    # padded stationary columns per block (fp8 DoubleRow
                           # needs 16-byte steps between K-tiles)

TRACE = False
LAST_RESULT = None

_COMPILED = {}
LDW_OPT = False            # walrus --enable-ldw-opt=true crashes codegen

_orig_run_command = bass_utils.run_command


def _run_command_ldw(argv, **kwargs):
    if LDW_OPT:
        argv = ["--enable-ldw-opt=true" if a == "--enable-ldw-opt=false"
                else a for a in argv]
    return _orig_run_command(argv, **kwargs)


bass_utils.run_command = _run_command_ldw

f32 = mybir.dt.float32
bf16 = mybir.dt.bfloat16
f8 = mybir.dt.float8e4
Alu = mybir.AluOpType
Act = mybir.ActivationFunctionType
bfnp = ml_dtypes.bfloat16
f8np = ml_dtypes.float8_e4m3


def _build():
    nc = bacc.Bacc("TRN2", target_bir_lowering=False, debug=False,
                   num_devices=N_CORES)

    yb_d = nc.dram_tensor("yb", [JB, NB], f32, kind="ExternalInput")
    w_d = nc.dram_tensor("W", [JB, NB * WK], f8, kind="ExternalInput")
    negfb_d = nc.dram_tensor("negfb", [JB, PC], bf16, kind="ExternalInput")
    negfT_d = nc.dram_tensor("negfT", [JB, NH], f32, kind="ExternalInput")
    uallT_d = nc.dram_tensor("uallT", [JB, NH], f32, kind="ExternalInput")
    uposT_d = nc.dram_tensor("uposT", [JB, NH], f32, kind="ExternalInput")
    corrb_d = nc.dram_tensor("corrb", [JB, 6], f32, kind="ExternalInput")
    out_d = nc.dram_tensor("out", [1, 1], f32, kind="ExternalOutput")

    with tile.TileContext(nc) as tc:
        with (
            tc.tile_pool(name="const", bufs=1) as cpool,
            tc.tile_pool(name="hpool", bufs=NB) as hpool,
            tc.tile_pool(name="psum", bufs=1, space="PSUM") as ppool,
            tc.tile_pool(name="small", bufs=1) as spool,
        ):
            y_f32 = cpool.tile([JB, NB], f32, name="y_f32")
            nc.sync.dma_start(y_f32[:], yb_d[:])
            negf_bf = cpool.tile([JB, PC], bf16, name="negf_bf")
            nc.sync.dma_start(negf_bf[:], negfb_d[:])
            W_all = cpool.tile([JB, NB * WK], f8, name="W_all")
            nc.sync.dma_start(W_all[:], w_d[:])

            negfT = spool.tile([JB, NH], f32, name="negfT")
            nc.sync.dma_start(negfT[:], negfT_d[:])
            uallT = spool.tile([JB, NH], f32, name="uallT")
            nc.sync.dma_start(uallT[:], uallT_d[:])
            uposT = spool.tile([JB, NH], f32, name="uposT")
            nc.sync.dma_start(uposT[:], uposT_d[:])
            corrb = spool.tile([JB, 6], f32, name="corrb")
            nc.sync.dma_start(corrb[:], corrb_d[:])

            ident = cpool.tile([JB, JB], f32, name="ident")
            make_identity(nc, ident)
            ones_f = cpool.tile([JB, 1], f32, name="ones_f")
            nc.vector.memset(ones_f[:], 1.0)

            # ---- H pass (into one fp8 tile; 0/1/-1 are exact in fp8) ----
            H_all = cpool.tile([JB, NB * PC], f8, name="H_all")
            for b in range(NB):
                hsl = H_all[:, b * PC:(b + 1) * PC]
                if b < DVE_BLOCKS:
                    nc.vector.tensor_scalar(hsl, negf_bf[:],
                                            y_f32[:, b:b + 1], 0.0,
                                            Alu.add, Alu.is_gt)
                else:
                    nc.scalar.activation(hsl, negf_bf[:], Act.Sign,
                                         bias=y_f32[:, b:b + 1])

            # ---- PE contraction: fp8 DoubleRow, 2 j-blocks per matmul ----
            psumH = ppool.tile([JB, PC], f32, name="psumH", tag="pg0")
            psumS = ppool.tile([JB, PC], f32, name="psumS", tag="pg1")
            w3d = W_all[:].rearrange("p (b k) -> p b k", k=WK)
            h3d = H_all[:].rearrange("p (b i) -> p b i", i=PC)
            NPAIR = NB // 2
            DPAIR = DVE_BLOCKS // 2
            # interleave the two sets so the PE can consume ACT-produced
            # blocks (ready early, in parallel) during DVE production gaps
            dve_q = list(range(DPAIR))
            act_q = list(range(DPAIR, NPAIR))
            order = []
            while dve_q or act_q:
                for _ in range(2):
                    if dve_q:
                        order.append(dve_q.pop(0))
                if act_q:
                    order.append(act_q.pop(0))
            first_seen = set()
            last_of = {True: max(p for p in order if p < DPAIR),
                       False: max(p for p in order if p >= DPAIR)}
            for pr in order:
                which = pr < DPAIR
                acc = psumH if which else psumS
                first = which not in first_seen
                first_seen.add(which)
                last = last_of[which] == pr
                nc.tensor.matmul(acc[0:WK, :],
                                 w3d[:, 2 * pr:2 * pr + 2, :],
                                 h3d[:, 2 * pr:2 * pr + 2, :],
                                 start=first, stop=last,
                                 perf_mode=mybir.MatmulPerfMode.DoubleRow,
                                 skip_group_check=True)

            Hsb = spool.tile([6, PC], f32, name="Hsb")
            nc.vector.tensor_copy(Hsb[:], psumH[0:6, :])
            Ssb = spool.tile([6, PC], f32, name="Ssb")
            nc.vector.tensor_copy(Ssb[:], psumS[0:6, :])

            contrib = spool.tile([JB, NH], f32, name="contrib")
            psumT = ppool.tile([JB, NH * 6], f32, name="psumT", tag="pg0")
            psumT2 = ppool.tile([JB, NH * 6], f32, name="psumT2", tag="pg1")
            for hh in range(NH):
                nc.tensor.transpose(psumT[:, hh * 6:(hh + 1) * 6],
                                    Hsb[:, hh * JB:(hh + 1) * JB],
                                    ident[0:6, 0:6])
                nc.tensor.transpose(psumT2[:, hh * 6:(hh + 1) * 6],
                                    Ssb[:, hh * JB:(hh + 1) * JB],
                                    ident[0:6, 0:6])

            for hh in range(NH):
                VH = spool.tile([JB, 6], f32, name=f"VH{hh}", tag="VH")
                nc.vector.tensor_copy(VH[:], psumT[:, hh * 6:(hh + 1) * 6])
                VS = spool.tile([JB, 6], f32, name=f"VS{hh}", tag="VS")
                nc.vector.tensor_copy(VS[:], psumT2[:, hh * 6:(hh + 1) * 6])

                # Vc = VH + 0.5*(VS + corr)  -> [A,B,C,Am,Bm,Cm]
                Vc = spool.tile([JB, 6], f32, name=f"Vc{hh}", tag="Vc")
                nc.vector.tensor_add(Vc[:], VS[:], corrb[:])
                nc.vector.scalar_tensor_tensor(Vc[:], Vc[:], 0.5, VH[:],
                                               Alu.mult, Alu.add)

                nf = negfT[:, hh:hh + 1]
                nf2 = spool.tile([JB, 1], f32, name=f"nf2{hh}", tag="nf2")
                nc.vector.tensor_mul(nf2[:], nf, nf)
                nf_2 = spool.tile([JB, 1], f32, name=f"nf_2{hh}", tag="nf_2")
                nc.vector.tensor_scalar(nf_2[:], nf, 2.0, 0.0,
                                        Alu.mult, Alu.add)

                # S = negf^2*A + (2negf*B + C); T likewise on masked cols
                S = spool.tile([JB, 1], f32, name=f"S{hh}", tag="S")
                nc.vector.scalar_tensor_tensor(S[:], Vc[:, 1:2], nf_2[:],
                                               Vc[:, 2:3], Alu.mult, Alu.add)
                nc.vector.scalar_tensor_tensor(S[:], Vc[:, 0:1], nf2[:],
                                               S[:], Alu.mult, Alu.add)
                T = spool.tile([JB, 1], f32, name=f"T{hh}", tag="T")
                nc.vector.scalar_tensor_tensor(T[:], Vc[:, 4:5], nf_2[:],
                                               Vc[:, 5:6], Alu.mult, Alu.add)
                nc.vector.scalar_tensor_tensor(T[:], Vc[:, 3:4], nf2[:],
                                               T[:], Alu.mult, Alu.add)

                # ua = (1-g)*uall + (g/N)*S ; up = (1-g)*upos + (g/N)*T
                ua = spool.tile([JB, 1], f32, name=f"ua{hh}", tag="ua")
                nc.vector.tensor_scalar(ua[:], uallT[:, hh:hh + 1],
                                        1.0 - GAMMA, 0.0, Alu.mult, Alu.add)
                nc.vector.scalar_tensor_tensor(ua[:], S[:], GAMMA / N, ua[:],
                                               Alu.mult, Alu.add)
                up = spool.tile([JB, 1], f32, name=f"up{hh}", tag="up")
                nc.vector.tensor_scalar(up[:], uposT[:, hh:hh + 1],
                                        1.0 - GAMMA, 0.0, Alu.mult, Alu.add)
                nc.vector.scalar_tensor_tensor(up[:], T[:], GAMMA / N, up[:],
                                               Alu.mult, Alu.add)

                inv = spool.tile([JB, 1], f32, name=f"inv{hh}", tag="inv")
                nc.vector.reciprocal(inv[:], ua[:])

                t1 = spool.tile([JB, 1], f32, name=f"t1{hh}", tag="t1")
                nc.vector.tensor_mul(t1[:], up[:], S[:])
                t2 = spool.tile([JB, 1], f32, name=f"t2{hh}", tag="t2")
                nc.vector.tensor_mul(t2[:], ua[:], T[:])
                d = spool.tile([JB, 1], f32, name=f"d{hh}", tag="d")
                nc.vector.tensor_sub(d[:], t1[:], t2[:])
                nc.vector.tensor_mul(d[:], d[:], inv[:])
                nc.vector.tensor_mul(d[:], d[:], inv[:])
                nc.vector.tensor_copy(contrib[:, hh:hh + 1], d[:])

            csum = spool.tile([JB, 1], f32, name="csum")
            nc.vector.tensor_add(csum[:], contrib[:, 0:1], contrib[:, 1:2])
            psum1 = ppool.tile([1, 1], f32, name="psum1", tag="pg2")
            nc.tensor.matmul(psum1[:], ones_f[:], csum[:], start=True,
                             stop=True)
            partial = spool.tile([1, 1], f32, name="partial")
            nc.vector.tensor_scalar(partial[:], psum1[:], INV_PN, 0.0,
                                    Alu.mult, Alu.add)
            nc.sync.dma_start(out_d[:], partial[:])

    nc.compile()
    return nc


def _host_w(yb: np.ndarray, maskb: np.ndarray):
    """W[p, b, :] = [1, y, y^2, m, m*y, m*y^2, 0...] in fp8e4 (WK cols)."""
    y = yb.astype(np.float32)
    y2 = (y * y).astype(np.float32)
    m = maskb.astype(np.float32)
    w = np.zeros((JB, NB, WK), dtype=np.float32)
    w[:, :, 0] = 1.0
    w[:, :, 1] = y
    w[:, :, 2] = y2
    w[:, :, 3] = m
    w[:, :, 4] = m * y
    w[:, :, 5] = m * y2
    wb = w.astype(f8np)
    return np.ascontiguousarray(wb.reshape(JB, NB * WK))


def kernel(y_pred, y_true, index_p, pos_idx, u_all, u_pos):
    global LAST_RESULT

    yp = np.asarray(y_pred, dtype=np.float32).reshape(-1)
    maskf = (np.asarray(y_true, dtype=np.float32).reshape(-1) == 1.0
             ).astype(np.float32)
    index_p = np.asarray(index_p).reshape(-1)
    pos_idx = np.asarray(pos_idx).reshape(-1)
    u_all_b = np.asarray(u_all, dtype=np.float32).reshape(-1)[index_p]
    u_pos_b = np.asarray(u_pos, dtype=np.float32).reshape(-1)[index_p]

    f_ps = yp[pos_idx]
    negf = (MARGIN - f_ps).astype(np.float32)       # (P,)

    nc = _COMPILED.get("nc")
    if nc is None:
        nc = _build()
        _COMPILED["nc"] = nc

    yb = np.ascontiguousarray(yp.reshape(NB, JB).T)
    maskb = np.ascontiguousarray(maskf.reshape(NB, JB).T)
    W = _host_w(yb, maskb)

    # sums of the (bf16-rounded) W columns over the ACT block range, for
    # the sign correction H.W = (Hs.W + sum(W))/2
    Wf = W.reshape(JB, NB, WK).astype(np.float64)
    corr = Wf[:, DVE_BLOCKS:, :6].sum(axis=(0, 1)).astype(np.float32)
    corrb = np.ascontiguousarray(
        np.broadcast_to(corr, (JB, 6))).astype(np.float32)

    in_maps = []
    for c in range(N_CORES):
        rs = slice(c * PC, (c + 1) * PC)
        negf_c = negf[rs]
        in_maps.append({
            "yb": yb,
            "W": W,
            "negfb": np.ascontiguousarray(
                np.broadcast_to(negf_c, (JB, PC))).astype(bfnp),
            "negfT": np.ascontiguousarray(negf_c.reshape(NH, JB).T),
            "uallT": np.ascontiguousarray(
                u_all_b[rs].reshape(NH, JB).T).astype(np.float32),
            "uposT": np.ascontiguousarray(
                u_pos_b[rs].reshape(NH, JB).T).astype(np.float32),
            "corrb": corrb,
        })

    res = bass_utils.run_bass_kernel_spmd(
        nc, in_maps, core_ids=list(range(N_CORES)), trace=TRACE)
    LAST_RESULT = res

    total = np.float32(0.0)
    for c in range(N_CORES):
        total = np.float32(total + res.results[c]["out"][0, 0])
    return np.asarray(total, dtype=np.float32)


# revision 14
# speedup vs baseline: 1.2055x; 1.0129x over previous
"""APLoss distributed Bass kernel for 8 TRN2 NeuronCores.

Reference math, restructured with an indicator decomposition:
    sur[i,j] = relu(t)^2,  t = negf_i + y_j,  negf_i = MARGIN - f_i
    relu(t)^2 = t^2 * H,   H = 1[t > 0]
    S_i = sum_j sur = negf_i^2 * A_i + 2*negf_i * B_i + C_i
      where A_i = sum_j H_ij, B_i = sum_j H_ij*y_j, C_i = sum_j H_ij*y_j^2
    T_i = masked version with (Am, Bm, Cm) using weights m_j*[1, y, y^2]
    ua_i = (1-g)*u_all[index_p[i]] + g*S_i/N
    up_i = (1-g)*u_pos[index_p[i]] + g*T_i/N
    loss = sum_i (up_i*S_i - ua_i*T_i) / ua_i^2 / (P*N)

Sharding: rows (positives) split 8 ways, 256 rows/core; y replicated.
Device layout: columns j on partitions (128 j-blocks of 128), rows i on
the free axis (256). Per core:
  DVE  (88 blocks): H = (negf + y_j) > 0           (tensor_scalar add,is_gt)
  ACT  (40 blocks): Hs = Sign(negf + y_j)          (activation, bias=y_j)
  PE: W_b^T @ H_b with W_b = [1, y, y^2, m, m*y, m*y^2] (host-built, bf16),
      4-way column-tiled (tile_position) into psumH/psumS row-groups at
      partitions {0,32,64,96}, accumulated over blocks.
  Sign-block sums corrected on device: H.W = (Hs.W + sum(W))/2 with sum(W)
  over the ACT column range passed as host constants.
  Finalize transposed to [128,2] (rows on partitions) for cheap vector ops;
  per-core scalar partial out; host sums the 8 partials.
"""

import sys

if "/opt/trn_rl_repo" not in sys.path:
    sys.path.insert(0, "/opt/trn_rl_repo")

import ml_dtypes
import numpy as np

import concourse.bass as bass
import concourse.tile as tile
from concourse import bacc, mybir
from concourse import bass_utils
from concourse.masks import make_identity

N = 16384
P = 2048
N_CORES = 8
PC = P // N_CORES          # rows per core (free dim)
JB = 128                   # j-block size (partitions)
NB = N // JB               # number of j-blocks
NH = PC // JB              # halves of the row range (2)
GAMMA = 0.99
MARGIN = 1.0
INV_PN = 1.0 / (P * N)     # 2^-25, exact

DVE_BLOCKS = 88            # H-blocks on the vector engine; rest on scalar (even)
WK = 16                    # padded stationary columns per block (fp8 DoubleRow
                           # needs 16-byte steps between K-tiles)

TRACE = False
LAST_RESULT = None

_COMPILED = {}
LDW_OPT = False            # walrus --enable-ldw-opt=true crashes codegen

_orig_run_command = bass_utils.run_command


def _run_command_ldw(argv, **kwargs):
    if LDW_OPT:
        argv = ["--enable-ldw-opt=true" if a == "--enable-ldw-opt=false"
                else a for a in argv]
    return _orig_run_command(argv, **kwargs)


bass_utils.run_command = _run_command_ldw

f32 = mybir.dt.float32
bf16 = mybir.dt.bfloat16
f8 = mybir.dt.float8e4
Alu = mybir.AluOpType
Act = mybir.ActivationFunctionType
bfnp = ml_dtypes.bfloat16
f8np = ml_dtypes.float8_e4m3


def _build():
    nc = bacc.Bacc("TRN2", target_bir_lowering=False, debug=False,
                   num_devices=N_CORES)

    yb_d = nc.dram_tensor("yb", [JB, NB], f32, kind="ExternalInput")
    w_d = nc.dram_tensor("W", [JB, NB * 6], bf16, kind="ExternalInput")
    negfb_d = nc.dram_tensor("negfb", [JB, PC], bf16, kind="ExternalInput")
    negfT_d = nc.dram_tensor("negfT", [JB, NH], f32, kind="ExternalInput")
    uallT_d = nc.dram_tensor("uallT", [JB, NH], f32, kind="ExternalInput")
    uposT_d = nc.dram_tensor("uposT", [JB, NH], f32, kind="ExternalInput")
    corrb_d = nc.dram_tensor("corrb", [JB, 6], f32, kind="ExternalInput")
    out_d = nc.dram_tensor("out", [1, 1], f32, kind="ExternalOutput")

    with tile.TileContext(nc) as tc:
        with (
            tc.tile_pool(name="const", bufs=1) as cpool,
            tc.tile_pool(name="hpool", bufs=NB) as hpool,
            tc.tile_pool(name="psum", bufs=1, space="PSUM") as ppool,
            tc.tile_pool(name="small", bufs=1) as spool,
        ):
            y_f32 = cpool.tile([JB, NB], f32, name="y_f32")
            nc.sync.dma_start(y_f32[:], yb_d[:])
            negf_bf = cpool.tile([JB, PC], bf16, name="negf_bf")
            nc.sync.dma_start(negf_bf[:], negfb_d[:])
            W_all = cpool.tile([JB, NB * 6], bf16, name="W_all")
            nc.sync.dma_start(W_all[:], w_d[:])

            negfT = spool.tile([JB, NH], f32, name="negfT")
            nc.sync.dma_start(negfT[:], negfT_d[:])
            uallT = spool.tile([JB, NH], f32, name="uallT")
            nc.sync.dma_start(uallT[:], uallT_d[:])
            uposT = spool.tile([JB, NH], f32, name="uposT")
            nc.sync.dma_start(uposT[:], uposT_d[:])
            corrb = spool.tile([JB, 6], f32, name="corrb")
            nc.sync.dma_start(corrb[:], corrb_d[:])

            ident = cpool.tile([JB, JB], f32, name="ident")
            make_identity(nc, ident)
            ones_f = cpool.tile([JB, 1], f32, name="ones_f")
            nc.vector.memset(ones_f[:], 1.0)

            # ---- PE warmup burst: keep HAM at full clock before the
            # real matmul stream begins (runs during the input DMAs) ----
            ones_bf = cpool.tile([JB, 1], bf16, name="ones_bf")
            nc.vector.memset(ones_bf[:], 1.0)
            wtile = cpool.tile([JB, PC], bf16, name="wtile")
            nc.vector.memset(wtile[:], 0.5)
            psumW = ppool.tile([1, PC], f32, name="psumW", tag="pg2")
            for _ in range(24):
                nc.tensor.matmul(psumW[:], ones_bf[:], wtile[:],
                                 start=True, stop=True)

            # ---- H pass ----
            h_tiles = []
            for b in range(NB):
                h = hpool.tile([JB, PC], bf16, name=f"h{b}", tag="h")
                if b < DVE_BLOCKS:
                    nc.vector.tensor_scalar(h[:], negf_bf[:],
                                            y_f32[:, b:b + 1], 0.0,
                                            Alu.add, Alu.is_gt)
                else:
                    nc.scalar.activation(h[:], negf_bf[:], Act.Sign,
                                         bias=y_f32[:, b:b + 1])
                h_tiles.append(h)

            # ---- PE contraction (interleave the two sets so the PE can
            # consume ACT-produced blocks during DVE production gaps) ----
            psumH = ppool.tile([JB, PC], f32, name="psumH", tag="pg0")
            psumS = ppool.tile([JB, PC], f32, name="psumS", tag="pg1")
            dve_q = list(range(DVE_BLOCKS))
            act_q = list(range(DVE_BLOCKS, NB))
            order = []
            while dve_q or act_q:
                for _ in range(2):
                    if dve_q:
                        order.append(dve_q.pop(0))
                if act_q:
                    order.append(act_q.pop(0))
            first_seen = set()
            last_of = {True: max(b for b in order if b < DVE_BLOCKS),
                       False: max(b for b in order if b >= DVE_BLOCKS)}
            for b in order:
                which = b < DVE_BLOCKS
                acc = psumH if which else psumS
                first = which not in first_seen
                first_seen.add(which)
                last = last_of[which] == b
                nc.tensor.matmul(acc[0:6, :],
                                 W_all[:, b * 6:(b + 1) * 6],
                                 h_tiles[b][:], start=first, stop=last,
                                 skip_group_check=True)

            Hsb = spool.tile([6, PC], f32, name="Hsb")
            nc.vector.tensor_copy(Hsb[:], psumH[0:6, :])
            Ssb = spool.tile([6, PC], f32, name="Ssb")
            nc.vector.tensor_copy(Ssb[:], psumS[0:6, :])

            contrib = spool.tile([JB, NH], f32, name="contrib")
            psumT = ppool.tile([JB, NH * 6], f32, name="psumT", tag="pg0")
            psumT2 = ppool.tile([JB, NH * 6], f32, name="psumT2", tag="pg1")
            for hh in range(NH):
                nc.tensor.transpose(psumT[:, hh * 6:(hh + 1) * 6],
                                    Hsb[:, hh * JB:(hh + 1) * JB],
                                    ident[0:6, 0:6])
                nc.tensor.transpose(psumT2[:, hh * 6:(hh + 1) * 6],
                                    Ssb[:, hh * JB:(hh + 1) * JB],
                                    ident[0:6, 0:6])

            for hh in range(NH):
                VH = spool.tile([JB, 6], f32, name=f"VH{hh}", tag="VH")
                nc.vector.tensor_copy(VH[:], psumT[:, hh * 6:(hh + 1) * 6])
                VS = spool.tile([JB, 6], f32, name=f"VS{hh}", tag="VS")
                nc.vector.tensor_copy(VS[:], psumT2[:, hh * 6:(hh + 1) * 6])

                # Vc = VH + 0.5*(VS + corr)  -> [A,B,C,Am,Bm,Cm]
                Vc = spool.tile([JB, 6], f32, name=f"Vc{hh}", tag="Vc")
                nc.vector.tensor_add(Vc[:], VS[:], corrb[:])
                nc.vector.scalar_tensor_tensor(Vc[:], Vc[:], 0.5, VH[:],
                                               Alu.mult, Alu.add)

                nf = negfT[:, hh:hh + 1]
                nf2 = spool.tile([JB, 1], f32, name=f"nf2{hh}", tag="nf2")
                nc.vector.tensor_mul(nf2[:], nf, nf)
                nf_2 = spool.tile([JB, 1], f32, name=f"nf_2{hh}", tag="nf_2")
                nc.vector.tensor_scalar(nf_2[:], nf, 2.0, 0.0,
                                        Alu.mult, Alu.add)

                # S = negf^2*A + (2negf*B + C); T likewise on masked cols
                S = spool.tile([JB, 1], f32, name=f"S{hh}", tag="S")
                nc.vector.scalar_tensor_tensor(S[:], Vc[:, 1:2], nf_2[:],
                                               Vc[:, 2:3], Alu.mult, Alu.add)
                nc.vector.scalar_tensor_tensor(S[:], Vc[:, 0:1], nf2[:],
                                               S[:], Alu.mult, Alu.add)
                T = spool.tile([JB, 1], f32, name=f"T{hh}", tag="T")
                nc.vector.scalar_tensor_tensor(T[:], Vc[:, 4:5], nf_2[:],
                                               Vc[:, 5:6], Alu.mult, Alu.add)
                nc.vector.scalar_tensor_tensor(T[:], Vc[:, 3:4], nf2[:],
                                               T[:], Alu.mult, Alu.add)

                # ua = (1-g)*uall + (g/N)*S ; up = (1-g)*upos + (g/N)*T
                ua = spool.tile([JB, 1], f32, name=f"ua{hh}", tag="ua")
                nc.vector.tensor_scalar(ua[:], uallT[:, hh:hh + 1],
                                        1.0 - GAMMA, 0.0, Alu.mult, Alu.add)
                nc.vector.scalar_tensor_tensor(ua[:], S[:], GAMMA / N, ua[:],
                                               Alu.mult, Alu.add)
                up = spool.tile([JB, 1], f32, name=f"up{hh}", tag="up")
                nc.vector.tensor_scalar(up[:], uposT[:, hh:hh + 1],
                                        1.0 - GAMMA, 0.0, Alu.mult, Alu.add)
                nc.vector.scalar_tensor_tensor(up[:], T[:], GAMMA / N, up[:],
                                               Alu.mult, Alu.add)

                inv = spool.tile([JB, 1], f32, name=f"inv{hh}", tag="inv")
                nc.vector.reciprocal(inv[:], ua[:])

                t1 = spool.tile([JB, 1], f32, name=f"t1{hh}", tag="t1")
                nc.vector.tensor_mul(t1[:], up[:], S[:])
                t2 = spool.tile([JB, 1], f32, name=f"t2{hh}", tag="t2")
                nc.vector.tensor_mul(t2[:], ua[:], T[:])
                d = spool.tile([JB, 1], f32, name=f"d{hh}", tag="d")
                nc.vector.tensor_sub(d[:], t1[:], t2[:])
                nc.vector.tensor_mul(d[:], d[:], inv[:])
                nc.vector.tensor_mul(d[:], d[:], inv[:])
                nc.vector.tensor_copy(contrib[:, hh:hh + 1], d[:])

            csum = spool.tile([JB, 1], f32, name="csum")
            nc.vector.tensor_add(csum[:], contrib[:, 0:1], contrib[:, 1:2])
            psum1 = ppool.tile([1, 1], f32, name="psum1", tag="pg2")
            nc.tensor.matmul(psum1[:], ones_f[:], csum[:], start=True,
                             stop=True)
            partial = spool.tile([1, 1], f32, name="partial")
            nc.vector.tensor_scalar(partial[:], psum1[:], INV_PN, 0.0,
                                    Alu.mult, Alu.add)
            nc.sync.dma_start(out_d[:], partial[:])

    nc.compile()
    return nc


def _host_w(yb: np.ndarray, maskb: np.ndarray):
    """W[p, b, :] = [1, y, y^2, m, m*y, m*y^2] in bf16."""
    y = yb.astype(np.float32)
    y2 = (y * y).astype(np.float32)
    m = maskb.astype(np.float32)
    w = np.stack([np.ones_like(y), y, y2, m, m * y, m * y2], axis=-1)
    wb = w.astype(bfnp)
    return np.ascontiguousarray(wb.reshape(JB, NB * 6))


def kernel(y_pred, y_true, index_p, pos_idx, u_all, u_pos):
    global LAST_RESULT

    yp = np.asarray(y_pred, dtype=np.float32).reshape(-1)
    maskf = (np.asarray(y_true, dtype=np.float32).reshape(-1) == 1.0
             ).astype(np.float32)
    index_p = np.asarray(index_p).reshape(-1)
    pos_idx = np.asarray(pos_idx).reshape(-1)
    u_all_b = np.asarray(u_all, dtype=np.float32).reshape(-1)[index_p]
    u_pos_b = np.asarray(u_pos, dtype=np.float32).reshape(-1)[index_p]

    f_ps = yp[pos_idx]
    negf = (MARGIN - f_ps).astype(np.float32)       # (P,)

    nc = _COMPILED.get("nc")
    if nc is None:
        nc = _build()
        _COMPILED["nc"] = nc

    yb = np.ascontiguousarray(yp.reshape(NB, JB).T)
    maskb = np.ascontiguousarray(maskf.reshape(NB, JB).T)
    W = _host_w(yb, maskb)

    # sums of the (bf16-rounded) W columns over the ACT block range, for
    # the sign correction H.W = (Hs.W + sum(W))/2
    Wf = W.reshape(JB, NB, 6).astype(np.float64)
    corr = Wf[:, DVE_BLOCKS:, :].sum(axis=(0, 1)).astype(np.float32)
    corrb = np.ascontiguousarray(
        np.broadcast_to(corr, (JB, 6))).astype(np.float32)

    in_maps = []
    for c in range(N_CORES):
        rs = slice(c * PC, (c + 1) * PC)
        negf_c = negf[rs]
        in_maps.append({
            "yb": yb,
            "W": W,
            "negfb": np.ascontiguousarray(
                np.broadcast_to(negf_c, (JB, PC))).astype(bfnp),
            "negfT": np.ascontiguousarray(negf_c.reshape(NH, JB).T),
            "uallT": np.ascontiguousarray(
                u_all_b[rs].reshape(NH, JB).T).astype(np.float32),
            "uposT": np.ascontiguousarray(
                u_pos_b[rs].reshape(NH, JB).T).astype(np.float32),
            "corrb": corrb,
        })

    res = bass_utils.run_bass_kernel_spmd(
        nc, in_maps, core_ids=list(range(N_CORES)), trace=TRACE)
    LAST_RESULT = res

    total = np.float32(0.0)
    for c in range(N_CORES):
        total = np.float32(total + res.results[c]["out"][0, 0])
    return np.asarray(total, dtype=np.float32)


# revision 16
# speedup vs baseline: 1.4197x; 1.1777x over previous
"""APLoss distributed Bass kernel for 8 TRN2 NeuronCores.

Reference math, restructured with an indicator decomposition:
    sur[i,j] = relu(t)^2,  t = negf_i + y_j,  negf_i = MARGIN - f_i
    relu(t)^2 = t^2 * H,   H = 1[t > 0]
    S_i = sum_j sur = negf_i^2 * A_i + 2*negf_i * B_i + C_i
      where A_i = sum_j H_ij, B_i = sum_j H_ij*y_j, C_i = sum_j H_ij*y_j^2
    T_i = masked version with (Am, Bm, Cm) using weights m_j*[1, y, y^2]
    ua_i = (1-g)*u_all[index_p[i]] + g*S_i/N
    up_i = (1-g)*u_pos[index_p[i]] + g*T_i/N
    loss = sum_i (up_i*S_i - ua_i*T_i) / ua_i^2 / (P*N)

Sharding: rows (positives) split 8 ways, 256 rows/core; y replicated.
Device layout: columns j on partitions (128 j-blocks of 128), rows i on
the free axis (256). Per core:
  DVE  (88 blocks): H = (negf + y_j) > 0           (tensor_scalar add,is_gt)
  ACT  (40 blocks): Hs = Sign(negf + y_j)          (activation, bias=y_j)
  PE: W_b^T @ H_b with W_b = [1, y, y^2, m, m*y, m*y^2] (host-built, bf16),
      4-way column-tiled (tile_position) into psumH/psumS row-groups at
      partitions {0,32,64,96}, accumulated over blocks.
  Sign-block sums corrected on device: H.W = (Hs.W + sum(W))/2 with sum(W)
  over the ACT column range passed as host constants.
  Finalize transposed to [128,2] (rows on partitions) for cheap vector ops;
  per-core scalar partial out; host sums the 8 partials.
"""

import sys

if "/opt/trn_rl_repo" not in sys.path:
    sys.path.insert(0, "/opt/trn_rl_repo")

import ml_dtypes
import numpy as np

import concourse.bass as bass
import concourse.tile as tile
from concourse import bacc, mybir
from concourse import bass_utils
from concourse.masks import make_identity
from concourse.tile_rust import add_dep_helper

N = 16384
P = 2048
N_CORES = 8
PC = P // N_CORES          # rows per core (free dim)
JB = 128                   # j-block size (partitions)
NB = N // JB               # number of j-blocks
NH = PC // JB              # halves of the row range (2)
GAMMA = 0.99
MARGIN = 1.0
INV_PN = 1.0 / (P * N)     # 2^-25, exact

DVE_BLOCKS = 88            # H-blocks on the vector engine; rest on scalar (even)
WK = 16                    # padded stationary columns per block (fp8 DoubleRow
                           # needs 16-byte steps between K-tiles)

TRACE = False
LAST_RESULT = None

_COMPILED = {}
LDW_OPT = False            # walrus --enable-ldw-opt=true crashes codegen

_orig_run_command = bass_utils.run_command


def _run_command_ldw(argv, **kwargs):
    if LDW_OPT:
        argv = ["--enable-ldw-opt=true" if a == "--enable-ldw-opt=false"
                else a for a in argv]
    return _orig_run_command(argv, **kwargs)


bass_utils.run_command = _run_command_ldw

f32 = mybir.dt.float32
bf16 = mybir.dt.bfloat16
f8 = mybir.dt.float8e4
Alu = mybir.AluOpType
Act = mybir.ActivationFunctionType
bfnp = ml_dtypes.bfloat16
f8np = ml_dtypes.float8_e4m3


def _build():
    nc = bacc.Bacc("TRN2", target_bir_lowering=False, debug=False,
                   num_devices=N_CORES)

    yb_d = nc.dram_tensor("yb", [JB, NB], f32, kind="ExternalInput")
    w_d = nc.dram_tensor("W", [JB, NB * 6], bf16, kind="ExternalInput")
    negfb_d = nc.dram_tensor("negfb", [JB, PC], bf16, kind="ExternalInput")
    negfT_d = nc.dram_tensor("negfT", [JB, NH], f32, kind="ExternalInput")
    uallT_d = nc.dram_tensor("uallT", [JB, NH], f32, kind="ExternalInput")
    uposT_d = nc.dram_tensor("uposT", [JB, NH], f32, kind="ExternalInput")
    corrb_d = nc.dram_tensor("corrb", [JB, 12], f32, kind="ExternalInput")
    out_d = nc.dram_tensor("out", [1, 1], f32, kind="ExternalOutput")

    with tile.TileContext(nc) as tc:
        with (
            tc.tile_pool(name="const", bufs=1) as cpool,
            tc.tile_pool(name="hpool", bufs=NB) as hpool,
            tc.tile_pool(name="psum", bufs=1, space="PSUM") as ppool,
            tc.tile_pool(name="small", bufs=1) as spool,
        ):
            y_f32 = cpool.tile([JB, NB], f32, name="y_f32")
            nc.sync.dma_start(y_f32[:], yb_d[:])
            negf_bf = cpool.tile([JB, PC], bf16, name="negf_bf")
            nc.sync.dma_start(negf_bf[:], negfb_d[:])
            W_all = cpool.tile([JB, NB * 6], bf16, name="W_all")
            nc.sync.dma_start(W_all[:], w_d[:])

            negfT = spool.tile([JB, NH], f32, name="negfT")
            nc.sync.dma_start(negfT[:], negfT_d[:])
            uallT = spool.tile([JB, NH], f32, name="uallT")
            nc.sync.dma_start(uallT[:], uallT_d[:])
            uposT = spool.tile([JB, NH], f32, name="uposT")
            nc.sync.dma_start(uposT[:], uposT_d[:])
            corrb = spool.tile([JB, 12], f32, name="corrb")
            nc.sync.dma_start(corrb[:], corrb_d[:])

            ident = cpool.tile([JB, JB], f32, name="ident")
            make_identity(nc, ident)
            ones_f = cpool.tile([JB, 1], f32, name="ones_f")
            nc.vector.memset(ones_f[:], 1.0)

            # ---- PE warmup burst: keep HAM at full clock before the
            # real matmul stream begins (runs during the input DMAs) ----
            ones_bf = cpool.tile([JB, 1], bf16, name="ones_bf")
            nc.vector.memset(ones_bf[:], 1.0)
            wtile = cpool.tile([JB, PC], bf16, name="wtile")
            nc.vector.memset(wtile[:], 0.5)
            psumW = ppool.tile([1, PC], f32, name="psumW", tag="pg2")
            for _ in range(24):
                nc.tensor.matmul(psumW[:], ones_bf[:], wtile[:],
                                 start=True, stop=True)

            # ---- H pass ----
            h_tiles = []
            for b in range(NB):
                h = hpool.tile([JB, PC], bf16, name=f"h{b}", tag="h")
                if b < DVE_BLOCKS:
                    nc.vector.tensor_scalar(h[:], negf_bf[:],
                                            y_f32[:, b:b + 1], 0.0,
                                            Alu.add, Alu.is_gt)
                else:
                    nc.scalar.activation(h[:], negf_bf[:], Act.Sign,
                                         bias=y_f32[:, b:b + 1])
                h_tiles.append(h)

            # ---- PE contraction (interleave the two sets so the PE can
            # consume ACT-produced blocks during DVE production gaps) ----
            psumH = ppool.tile([JB, PC], f32, name="psumH", tag="pg0")
            psumS = ppool.tile([JB, PC], f32, name="psumS", tag="pg1")
            # merge-sort the two sets by predicted H readiness (DVE
            # ~197ns/block, ACT ~402ns/block) so the PE's strict-FIFO
            # queue never head-blocks on an unproduced tile
            ready = [(197.0 * (b + 1), b) for b in range(DVE_BLOCKS)]
            ready += [(402.0 * (b - DVE_BLOCKS + 1) + 5.0, b)
                      for b in range(DVE_BLOCKS, NB)]
            order = [b for _, b in sorted(ready)]
            first_seen = set()
            last_of = {True: max(b for b in order if b < DVE_BLOCKS),
                       False: max(b for b in order if b >= DVE_BLOCKS)}
            prev_mm = None
            for b in order:
                which = b < DVE_BLOCKS
                acc = psumH if which else psumS
                first = which not in first_seen
                first_seen.add(which)
                last = last_of[which] == b
                mm = nc.tensor.matmul(acc[0:6, :],
                                      W_all[:, b * 6:(b + 1) * 6],
                                      h_tiles[b][:], start=first, stop=last,
                                      skip_group_check=True)
                if prev_mm is not None:
                    add_dep_helper(mm.ins, prev_mm.ins,
                                   reason="keep PE consumption order")
                prev_mm = mm

            Hsb = spool.tile([6, PC], f32, name="Hsb")
            nc.vector.tensor_copy(Hsb[:], psumH[0:6, :])
            Ssb = spool.tile([6, PC], f32, name="Ssb")
            nc.vector.tensor_copy(Ssb[:], psumS[0:6, :])

            psumT = ppool.tile([JB, NH * 6], f32, name="psumT", tag="pg0")
            psumT2 = ppool.tile([JB, NH * 6], f32, name="psumT2", tag="pg1")
            for hh in range(NH):
                nc.tensor.transpose(psumT[:, hh * 6:(hh + 1) * 6],
                                    Hsb[:, hh * JB:(hh + 1) * JB],
                                    ident[0:6, 0:6])
                nc.tensor.transpose(psumT2[:, hh * 6:(hh + 1) * 6],
                                    Ssb[:, hh * JB:(hh + 1) * JB],
                                    ident[0:6, 0:6])

            # finalize on [128, 2] tiles: 256 rows on partitions, both
            # halves as the two free columns
            VH = spool.tile([JB, 12], f32, name="VH")
            nc.vector.tensor_copy(VH[:], psumT[:])
            VS = spool.tile([JB, 12], f32, name="VS")
            nc.vector.tensor_copy(VS[:], psumT2[:])
            Vc = spool.tile([JB, 12], f32, name="Vc")
            nc.vector.tensor_add(Vc[:], VS[:], corrb[:])
            nc.vector.scalar_tensor_tensor(Vc[:], Vc[:], 0.5, VH[:],
                                           Alu.mult, Alu.add)
            v = Vc[:].rearrange("p (h k) -> p h k", k=6)
            A2, B2, C2 = v[:, :, 0], v[:, :, 1], v[:, :, 2]
            Am2, Bm2, Cm2 = v[:, :, 3], v[:, :, 4], v[:, :, 5]

            nf2 = spool.tile([JB, NH], f32, name="nf2")
            nc.vector.tensor_mul(nf2[:], negfT[:], negfT[:])
            n2 = spool.tile([JB, NH], f32, name="n2")
            nc.vector.tensor_scalar(n2[:], negfT[:], 2.0, 0.0,
                                    Alu.mult, Alu.add)

            S2 = spool.tile([JB, NH], f32, name="S2")
            nc.vector.tensor_mul(S2[:], B2, n2[:])
            nc.vector.tensor_add(S2[:], S2[:], C2)
            t2a = spool.tile([JB, NH], f32, name="t2a")
            nc.vector.tensor_mul(t2a[:], A2, nf2[:])
            nc.vector.tensor_add(S2[:], S2[:], t2a[:])

            T2 = spool.tile([JB, NH], f32, name="T2")
            nc.vector.tensor_mul(T2[:], Bm2, n2[:])
            nc.vector.tensor_add(T2[:], T2[:], Cm2)
            nc.vector.tensor_mul(t2a[:], Am2, nf2[:])
            nc.vector.tensor_add(T2[:], T2[:], t2a[:])

            ua2 = spool.tile([JB, NH], f32, name="ua2")
            nc.vector.tensor_scalar(ua2[:], uallT[:], 1.0 - GAMMA, 0.0,
                                    Alu.mult, Alu.add)
            nc.vector.scalar_tensor_tensor(ua2[:], S2[:], GAMMA / N, ua2[:],
                                           Alu.mult, Alu.add)
            up2 = spool.tile([JB, NH], f32, name="up2")
            nc.vector.tensor_scalar(up2[:], uposT[:], 1.0 - GAMMA, 0.0,
                                    Alu.mult, Alu.add)
            nc.vector.scalar_tensor_tensor(up2[:], T2[:], GAMMA / N, up2[:],
                                           Alu.mult, Alu.add)

            inv2 = spool.tile([JB, NH], f32, name="inv2")
            nc.vector.reciprocal(inv2[:], ua2[:])

            d1 = spool.tile([JB, NH], f32, name="d1")
            nc.vector.tensor_mul(d1[:], up2[:], S2[:])
            d2 = spool.tile([JB, NH], f32, name="d2")
            nc.vector.tensor_mul(d2[:], ua2[:], T2[:])
            nc.vector.tensor_sub(d1[:], d1[:], d2[:])
            nc.vector.tensor_mul(d1[:], d1[:], inv2[:])
            nc.vector.tensor_mul(d1[:], d1[:], inv2[:])

            csum = spool.tile([JB, 1], f32, name="csum")
            nc.vector.tensor_add(csum[:], d1[:, 0:1], d1[:, 1:2])
            psum1 = ppool.tile([1, 1], f32, name="psum1", tag="pg2")
            nc.tensor.matmul(psum1[:], ones_f[:], csum[:], start=True,
                             stop=True)
            partial = spool.tile([1, 1], f32, name="partial")
            nc.vector.tensor_scalar(partial[:], psum1[:], INV_PN, 0.0,
                                    Alu.mult, Alu.add)
            nc.sync.dma_start(out_d[:], partial[:])

    nc.compile()
    return nc


def _host_w(yb: np.ndarray, maskb: np.ndarray):
    """W[p, b, :] = [1, y, y^2, m, m*y, m*y^2] in bf16."""
    y = yb.astype(np.float32)
    y2 = (y * y).astype(np.float32)
    m = maskb.astype(np.float32)
    w = np.stack([np.ones_like(y), y, y2, m, m * y, m * y2], axis=-1)
    wb = w.astype(bfnp)
    return np.ascontiguousarray(wb.reshape(JB, NB * 6))


def kernel(y_pred, y_true, index_p, pos_idx, u_all, u_pos):
    global LAST_RESULT

    yp = np.asarray(y_pred, dtype=np.float32).reshape(-1)
    maskf = (np.asarray(y_true, dtype=np.float32).reshape(-1) == 1.0
             ).astype(np.float32)
    index_p = np.asarray(index_p).reshape(-1)
    pos_idx = np.asarray(pos_idx).reshape(-1)
    u_all_b = np.asarray(u_all, dtype=np.float32).reshape(-1)[index_p]
    u_pos_b = np.asarray(u_pos, dtype=np.float32).reshape(-1)[index_p]

    f_ps = yp[pos_idx]
    negf = (MARGIN - f_ps).astype(np.float32)       # (P,)

    nc = _COMPILED.get("nc")
    if nc is None:
        nc = _build()
        _COMPILED["nc"] = nc

    yb = np.ascontiguousarray(yp.reshape(NB, JB).T)
    maskb = np.ascontiguousarray(maskf.reshape(NB, JB).T)
    W = _host_w(yb, maskb)

    # sums of the (bf16-rounded) W columns over the ACT block range, for
    # the sign correction H.W = (Hs.W + sum(W))/2
    Wf = W.reshape(JB, NB, 6).astype(np.float64)
    corr = Wf[:, DVE_BLOCKS:, :].sum(axis=(0, 1)).astype(np.float32)
    corr12 = np.concatenate([corr, corr])  # [A,B,C,Am,Bm,Cm] x 2 halves
    corrb = np.ascontiguousarray(
        np.broadcast_to(corr12, (JB, 12))).astype(np.float32)

    in_maps = []
    for c in range(N_CORES):
        rs = slice(c * PC, (c + 1) * PC)
        negf_c = negf[rs]
        in_maps.append({
            "yb": yb,
            "W": W,
            "negfb": np.ascontiguousarray(
                np.broadcast_to(negf_c, (JB, PC))).astype(bfnp),
            "negfT": np.ascontiguousarray(negf_c.reshape(NH, JB).T),
            "uallT": np.ascontiguousarray(
                u_all_b[rs].reshape(NH, JB).T).astype(np.float32),
            "uposT": np.ascontiguousarray(
                u_pos_b[rs].reshape(NH, JB).T).astype(np.float32),
            "corrb": corrb,
        })

    res = bass_utils.run_bass_kernel_spmd(
        nc, in_maps, core_ids=list(range(N_CORES)), trace=TRACE)
    LAST_RESULT = res

    total = np.float32(0.0)
    for c in range(N_CORES):
        total = np.float32(total + res.results[c]["out"][0, 0])
    return np.asarray(total, dtype=np.float32)


# revision 17
# speedup vs baseline: 1.4299x; 1.0072x over previous
"""APLoss distributed Bass kernel for 8 TRN2 NeuronCores.

Reference math, restructured with an indicator decomposition:
    sur[i,j] = relu(t)^2,  t = negf_i + y_j,  negf_i = MARGIN - f_i
    relu(t)^2 = t^2 * H,   H = 1[t > 0]
    S_i = sum_j sur = negf_i^2 * A_i + 2*negf_i * B_i + C_i
      where A_i = sum_j H_ij, B_i = sum_j H_ij*y_j, C_i = sum_j H_ij*y_j^2
    T_i = masked version with (Am, Bm, Cm) using weights m_j*[1, y, y^2]
    ua_i = (1-g)*u_all[index_p[i]] + g*S_i/N
    up_i = (1-g)*u_pos[index_p[i]] + g*T_i/N
    loss = sum_i (up_i*S_i - ua_i*T_i) / ua_i^2 / (P*N)

Sharding: rows (positives) split 8 ways, 256 rows/core; y replicated.
Device layout: columns j on partitions (128 j-blocks of 128), rows i on
the free axis (256). Per core:
  DVE  (88 blocks): H = (negf + y_j) > 0           (tensor_scalar add,is_gt)
  ACT  (40 blocks): Hs = Sign(negf + y_j)          (activation, bias=y_j)
  PE: W_b^T @ H_b with W_b = [1, y, y^2, m, m*y, m*y^2] (host-built, bf16),
      4-way column-tiled (tile_position) into psumH/psumS row-groups at
      partitions {0,32,64,96}, accumulated over blocks.
  Sign-block sums corrected on device: H.W = (Hs.W + sum(W))/2 with sum(W)
  over the ACT column range passed as host constants.
  Finalize transposed to [128,2] (rows on partitions) for cheap vector ops;
  per-core scalar partial out; host sums the 8 partials.
"""

import sys

if "/opt/trn_rl_repo" not in sys.path:
    sys.path.insert(0, "/opt/trn_rl_repo")

import ml_dtypes
import numpy as np

import concourse.bass as bass
import concourse.tile as tile
from concourse import bacc, mybir
from concourse import bass_utils
from concourse.masks import make_identity
from concourse.tile_rust import add_dep_helper

N = 16384
P = 2048
N_CORES = 8
PC = P // N_CORES          # rows per core (free dim)
JB = 128                   # j-block size (partitions)
NB = N // JB               # number of j-blocks
NH = PC // JB              # halves of the row range (2)
GAMMA = 0.99
MARGIN = 1.0
INV_PN = 1.0 / (P * N)     # 2^-25, exact

DVE_BLOCKS = 88            # H-blocks on the vector engine; rest on scalar (even)
WK = 16                    # padded stationary columns per block (fp8 DoubleRow
                           # needs 16-byte steps between K-tiles)

TRACE = False
LAST_RESULT = None

_COMPILED = {}
LDW_OPT = False            # walrus --enable-ldw-opt=true crashes codegen

_orig_run_command = bass_utils.run_command


def _run_command_ldw(argv, **kwargs):
    if LDW_OPT:
        argv = ["--enable-ldw-opt=true" if a == "--enable-ldw-opt=false"
                else a for a in argv]
    return _orig_run_command(argv, **kwargs)


bass_utils.run_command = _run_command_ldw

f32 = mybir.dt.float32
bf16 = mybir.dt.bfloat16
f8 = mybir.dt.float8e4
Alu = mybir.AluOpType
Act = mybir.ActivationFunctionType
bfnp = ml_dtypes.bfloat16
f8np = ml_dtypes.float8_e4m3


def _build():
    nc = bacc.Bacc("TRN2", target_bir_lowering=False, debug=False,
                   num_devices=N_CORES)

    yb_d = nc.dram_tensor("yb", [JB, NB], f32, kind="ExternalInput")
    w_d = nc.dram_tensor("W", [JB, NB * 6], bf16, kind="ExternalInput")
    negfb_d = nc.dram_tensor("negfb", [JB, PC], bf16, kind="ExternalInput")
    negfT_d = nc.dram_tensor("negfT", [JB, NH], f32, kind="ExternalInput")
    uallT_d = nc.dram_tensor("uallT", [JB, NH], f32, kind="ExternalInput")
    uposT_d = nc.dram_tensor("uposT", [JB, NH], f32, kind="ExternalInput")
    corrb_d = nc.dram_tensor("corrb", [JB, 12], f32, kind="ExternalInput")
    out_d = nc.dram_tensor("out", [1, 1], f32, kind="ExternalOutput")

    with tile.TileContext(nc) as tc:
        with (
            tc.tile_pool(name="const", bufs=1) as cpool,
            tc.tile_pool(name="hpool", bufs=NB) as hpool,
            tc.tile_pool(name="psum", bufs=1, space="PSUM") as ppool,
            tc.tile_pool(name="small", bufs=1) as spool,
        ):
            y_f32 = cpool.tile([JB, NB], f32, name="y_f32")
            nc.sync.dma_start(y_f32[:], yb_d[:])
            negf_bf = cpool.tile([JB, PC], bf16, name="negf_bf")
            nc.sync.dma_start(negf_bf[:], negfb_d[:])
            W_all = cpool.tile([JB, NB * 6], bf16, name="W_all")
            nc.sync.dma_start(W_all[:], w_d[:])

            negfT = spool.tile([JB, NH], f32, name="negfT")
            nc.sync.dma_start(negfT[:], negfT_d[:])
            uallT = spool.tile([JB, NH], f32, name="uallT")
            nc.sync.dma_start(uallT[:], uallT_d[:])
            uposT = spool.tile([JB, NH], f32, name="uposT")
            nc.sync.dma_start(uposT[:], uposT_d[:])
            corrb = spool.tile([JB, 12], f32, name="corrb")
            nc.sync.dma_start(corrb[:], corrb_d[:])

            ident = cpool.tile([JB, JB], f32, name="ident")
            make_identity(nc, ident)
            ones_f = cpool.tile([JB, 1], f32, name="ones_f")
            nc.vector.memset(ones_f[:], 1.0)

            # ---- PE warmup burst: keep HAM at full clock before the
            # real matmul stream begins (runs during the input DMAs) ----
            ones_bf = cpool.tile([JB, 1], bf16, name="ones_bf")
            nc.vector.memset(ones_bf[:], 1.0)
            wtile = cpool.tile([JB, PC], bf16, name="wtile")
            nc.vector.memset(wtile[:], 0.5)
            psumW = ppool.tile([1, PC], f32, name="psumW", tag="pg2")
            for _ in range(24):
                nc.tensor.matmul(psumW[:], ones_bf[:], wtile[:],
                                 start=True, stop=True)

            # ---- H pass ----
            h_tiles = []
            for b in range(NB):
                h = hpool.tile([JB, PC], bf16, name=f"h{b}", tag="h")
                if b < DVE_BLOCKS:
                    nc.vector.tensor_scalar(h[:], negf_bf[:],
                                            y_f32[:, b:b + 1], 0.0,
                                            Alu.add, Alu.is_gt)
                else:
                    nc.scalar.activation(h[:], negf_bf[:], Act.Sign,
                                         bias=y_f32[:, b:b + 1])
                h_tiles.append(h)

            # ---- PE contraction (interleave the two sets so the PE can
            # consume ACT-produced blocks during DVE production gaps) ----
            psumH = ppool.tile([JB, PC], f32, name="psumH", tag="pg0")
            psumS = ppool.tile([JB, PC], f32, name="psumS", tag="pg1")
            # merge-sort the two sets by predicted H readiness (DVE
            # ~197ns/block, ACT ~402ns/block) so the PE's strict-FIFO
            # queue never head-blocks on an unproduced tile
            ready = [(197.0 * (b + 1), b) for b in range(DVE_BLOCKS)]
            ready += [(402.0 * (b - DVE_BLOCKS + 1) + 5.0, b)
                      for b in range(DVE_BLOCKS, NB)]
            order = [b for _, b in sorted(ready)]
            first_seen = set()
            last_of = {True: max(b for b in order if b < DVE_BLOCKS),
                       False: max(b for b in order if b >= DVE_BLOCKS)}
            prev_mm = None
            for b in order:
                which = b < DVE_BLOCKS
                acc = psumH if which else psumS
                first = which not in first_seen
                first_seen.add(which)
                last = last_of[which] == b
                mm = nc.tensor.matmul(acc[0:6, :],
                                      W_all[:, b * 6:(b + 1) * 6],
                                      h_tiles[b][:], start=first, stop=last,
                                      skip_group_check=True)
                if prev_mm is not None:
                    add_dep_helper(mm.ins, prev_mm.ins,
                                   reason="keep PE consumption order")
                prev_mm = mm

            Hsb = spool.tile([6, PC], f32, name="Hsb")
            nc.vector.tensor_copy(Hsb[:], psumH[0:6, :])
            Ssb = spool.tile([6, PC], f32, name="Ssb")
            nc.scalar.copy(Ssb[:], psumS[0:6, :])

            psumT = ppool.tile([JB, NH * 6], f32, name="psumT", tag="pg0")
            psumT2 = ppool.tile([JB, NH * 6], f32, name="psumT2", tag="pg1")
            for hh in range(NH):
                nc.tensor.transpose(psumT[:, hh * 6:(hh + 1) * 6],
                                    Hsb[:, hh * JB:(hh + 1) * JB],
                                    ident[0:6, 0:6])
                nc.tensor.transpose(psumT2[:, hh * 6:(hh + 1) * 6],
                                    Ssb[:, hh * JB:(hh + 1) * JB],
                                    ident[0:6, 0:6])

            # finalize on [128, 2] tiles: 256 rows on partitions, both
            # halves as the two free columns
            VH = spool.tile([JB, 12], f32, name="VH")
            nc.vector.tensor_copy(VH[:], psumT[:])
            VS = spool.tile([JB, 12], f32, name="VS")
            nc.vector.tensor_copy(VS[:], psumT2[:])
            Vc = spool.tile([JB, 12], f32, name="Vc")
            nc.vector.tensor_add(Vc[:], VS[:], corrb[:])
            nc.vector.scalar_tensor_tensor(Vc[:], Vc[:], 0.5, VH[:],
                                           Alu.mult, Alu.add)
            v = Vc[:].rearrange("p (h k) -> p h k", k=6)
            A2, B2, C2 = v[:, :, 0], v[:, :, 1], v[:, :, 2]
            Am2, Bm2, Cm2 = v[:, :, 3], v[:, :, 4], v[:, :, 5]

            nf2 = spool.tile([JB, NH], f32, name="nf2")
            nc.vector.tensor_mul(nf2[:], negfT[:], negfT[:])
            n2 = spool.tile([JB, NH], f32, name="n2")
            nc.vector.tensor_scalar(n2[:], negfT[:], 2.0, 0.0,
                                    Alu.mult, Alu.add)

            S2 = spool.tile([JB, NH], f32, name="S2")
            nc.vector.tensor_mul(S2[:], B2, n2[:])
            nc.vector.tensor_add(S2[:], S2[:], C2)
            t2a = spool.tile([JB, NH], f32, name="t2a")
            nc.vector.tensor_mul(t2a[:], A2, nf2[:])
            nc.vector.tensor_add(S2[:], S2[:], t2a[:])

            T2 = spool.tile([JB, NH], f32, name="T2")
            nc.vector.tensor_mul(T2[:], Bm2, n2[:])
            nc.vector.tensor_add(T2[:], T2[:], Cm2)
            nc.vector.tensor_mul(t2a[:], Am2, nf2[:])
            nc.vector.tensor_add(T2[:], T2[:], t2a[:])

            ua2 = spool.tile([JB, NH], f32, name="ua2")
            nc.vector.tensor_scalar(ua2[:], uallT[:], 1.0 - GAMMA, 0.0,
                                    Alu.mult, Alu.add)
            nc.vector.scalar_tensor_tensor(ua2[:], S2[:], GAMMA / N, ua2[:],
                                           Alu.mult, Alu.add)
            up2 = spool.tile([JB, NH], f32, name="up2")
            nc.vector.tensor_scalar(up2[:], uposT[:], 1.0 - GAMMA, 0.0,
                                    Alu.mult, Alu.add)
            nc.vector.scalar_tensor_tensor(up2[:], T2[:], GAMMA / N, up2[:],
                                           Alu.mult, Alu.add)

            inv2 = spool.tile([JB, NH], f32, name="inv2")
            nc.vector.reciprocal(inv2[:], ua2[:])

            d1 = spool.tile([JB, NH], f32, name="d1")
            nc.vector.tensor_mul(d1[:], up2[:], S2[:])
            d2 = spool.tile([JB, NH], f32, name="d2")
            nc.vector.tensor_mul(d2[:], ua2[:], T2[:])
            nc.vector.tensor_sub(d1[:], d1[:], d2[:])
            nc.vector.tensor_mul(d1[:], d1[:], inv2[:])
            nc.vector.tensor_mul(d1[:], d1[:], inv2[:])

            csum = spool.tile([JB, 1], f32, name="csum")
            nc.vector.tensor_add(csum[:], d1[:, 0:1], d1[:, 1:2])
            psum1 = ppool.tile([1, 1], f32, name="psum1", tag="pg2")
            nc.tensor.matmul(psum1[:], ones_f[:], csum[:], start=True,
                             stop=True)
            partial = spool.tile([1, 1], f32, name="partial")
            nc.vector.tensor_scalar(partial[:], psum1[:], INV_PN, 0.0,
                                    Alu.mult, Alu.add)
            nc.sync.dma_start(out_d[:], partial[:])

    nc.compile()
    return nc


def _host_w(yb: np.ndarray, maskb: np.ndarray):
    """W[p, b, :] = [1, y, y^2, m, m*y, m*y^2] in bf16."""
    y = yb.astype(np.float32)
    y2 = (y * y).astype(np.float32)
    m = maskb.astype(np.float32)
    w = np.stack([np.ones_like(y), y, y2, m, m * y, m * y2], axis=-1)
    wb = w.astype(bfnp)
    return np.ascontiguousarray(wb.reshape(JB, NB * 6))


def kernel(y_pred, y_true, index_p, pos_idx, u_all, u_pos):
    global LAST_RESULT

    yp = np.asarray(y_pred, dtype=np.float32).reshape(-1)
    maskf = (np.asarray(y_true, dtype=np.float32).reshape(-1) == 1.0
             ).astype(np.float32)
    index_p = np.asarray(index_p).reshape(-1)
    pos_idx = np.asarray(pos_idx).reshape(-1)
    u_all_b = np.asarray(u_all, dtype=np.float32).reshape(-1)[index_p]
    u_pos_b = np.asarray(u_pos, dtype=np.float32).reshape(-1)[index_p]

    f_ps = yp[pos_idx]
    negf = (MARGIN - f_ps).astype(np.float32)       # (P,)

    nc = _COMPILED.get("nc")
    if nc is None:
        nc = _build()
        _COMPILED["nc"] = nc

    yb = np.ascontiguousarray(yp.reshape(NB, JB).T)
    maskb = np.ascontiguousarray(maskf.reshape(NB, JB).T)
    W = _host_w(yb, maskb)

    # sums of the (bf16-rounded) W columns over the ACT block range, for
    # the sign correction H.W = (Hs.W + sum(W))/2
    Wf = W.reshape(JB, NB, 6).astype(np.float64)
    corr = Wf[:, DVE_BLOCKS:, :].sum(axis=(0, 1)).astype(np.float32)
    corr12 = np.concatenate([corr, corr])  # [A,B,C,Am,Bm,Cm] x 2 halves
    corrb = np.ascontiguousarray(
        np.broadcast_to(corr12, (JB, 12))).astype(np.float32)

    in_maps = []
    for c in range(N_CORES):
        rs = slice(c * PC, (c + 1) * PC)
        negf_c = negf[rs]
        in_maps.append({
            "yb": yb,
            "W": W,
            "negfb": np.ascontiguousarray(
                np.broadcast_to(negf_c, (JB, PC))).astype(bfnp),
            "negfT": np.ascontiguousarray(negf_c.reshape(NH, JB).T),
            "uallT": np.ascontiguousarray(
                u_all_b[rs].reshape(NH, JB).T).astype(np.float32),
            "uposT": np.ascontiguousarray(
                u_pos_b[rs].reshape(NH, JB).T).astype(np.float32),
            "corrb": corrb,
        })

    res = bass_utils.run_bass_kernel_spmd(
        nc, in_maps, core_ids=list(range(N_CORES)), trace=TRACE)
    LAST_RESULT = res

    total = np.float32(0.0)
    for c in range(N_CORES):
        total = np.float32(total + res.results[c]["out"][0, 0])
    return np.asarray(total, dtype=np.float32)
